# revision 73
# baseline (speedup 1.0000x reference)
"""Multi-head self-attention block (B=4, N=2048, D=384, H=8, FF=1536) on 8 TRN2 cores.

Sharding: data-parallel over tokens. Core c handles batch b=c//2, query rows
[(c%2)*1024, (c%2+1)*1024). K/V are computed per-batch on each core (2x
replicated work, zero collectives).

v2: ALL projections and the scores matmul run fp8(e4m3) in
DoubleRowSwInterleave perf mode (2 contraction rows per partition, 0.5
cycles/row). Q and K are produced in pair-interleaved fp8 layouts so the
scores matmul S = K^T Q itself runs DR: per head, contraction is 48 dims =
24 pair-partitions; Q is the DR ifmap (q8[g][24*ix+p, 2n+s] = Q[dim 2p+s,
query n], natural column order) and K the DR weights (k8r, SwInterleave
layout = pair-interleaved + column-reversed per 128-key block). Both are
produced by parity-split projections: even/odd head dims as separate
128-out-partition DR matmuls (host-arranged weight columns wq8x/wk8x, 96
real + 32 junk partitions), whose PSUM tiles are evacuated with stride-2
column writes into the shared q8/k8r tiles. The key reversal for k8r comes
free: kproj's ifmap is y8rT (column-reversed per 128-block), which already
exists as the vproj weights input, so the natural-order y8T input is gone.
qt (fp16 feature-major, for the Q residual) is produced by a separate DR
projection from the same x8 ifmap with padded-head-layout weights wq8pad.

Head padding: each 48-dim head occupies a 64-slot block:
  slots 0-47 = head dims, slot 48 = softmax-denominator slot, 49-63 = junk.
V is row-major "augmented": vaug[j] = [128 keys, 8*64] with per-head block
cols [V dims 0-47 | 1.0 | junk]; the ones column makes the P@V matmul drop
the softmax denominator into output col 48.

Attention datapath per head pair t (heads 2t, 2t+1):
  scores  S[j-tile, q] = K^T Q     (fp8 DR; PSUM f32 [128,1024], keys on
          partitions)
  exp     head A tiles: ACT Exp; head B tiles: DVE Schraudolph bit-trick
          (out_i16 = s*A16 + B16, bitcast fp16, ~2%% rms exp error that
          largely cancels between softmax numerator and denominator) --
          splitting softmax exp across both engines is what keeps either
          from being the bottleneck; it is the largest single evacuation load.
  P@V     TRANSPOSED: out[q, v] = sum_j P[j,q] V[j,v] -- queries on output
          partitions (full 128-wide PE use; 49-wide moving dim). 8 i-tile
          accumulators per head packed at 64-col offsets into one PSUM bank.
          No memset: the first accumulator's j=0 matmul uses start=True,
          which zeroes its entire 2KB bank in hardware (per-start bank
          zeroing clobbers neighbors, so ONLY i==0 starts); the other seven
          accumulate onto the zeroed bank with start=False.
  norm    denominator is per-partition (col 48): one batched DVE reciprocal
          per head ([128,8] over the stride-64 denominator columns), then
          ACT activation-Copy with per-partition scale (head A) / DVE
          tensor_scalar (head B) into o_r [128 q, 128] fp16.
  back    one PE transpose per (t, i) -> [128 v, 128 q] fp16 via bitcast
          views into a [128,1024] PSUM tile (4 transposes per tile, one per
          2KB bank so start-zeroing cannot clobber), then a single DVE
          scalar_tensor_tensor per 4 i-tiles adds the Q residual while
          copying to the padded feature-major ot_p.
ot_p is compacted 512->384 rows by 10 partition-moving SBUF->SBUF DMAs
(issued from the gpsimd sequencer), then the FFN (fp16 weights, f32 PSUM)
runs over compact dims: FFN2 accumulators m=0,1 are fed per-gelu; m=2
accumulates from retained hf tiles afterward.

Quirks: gpsimd ALU/memset ops touching PSUM fail neuronxcc codegen (gpsimd
does DMA issue + SBUF memsets only); DMA cannot read PSUM; plain DoubleRow
and nonzero tile_position columns are invalid ISA here; the Tile scheduler
reorders by readiness, so emission order is a hint. Input DMAs are spread
across the SP/ACT/gpsimd sequencers so the critical kproj/qproj inputs land
early.
"""

import math
import os
import numpy as np

# schedule knobs (sweepable via env for tuning; defaults = tuned values)
DRAIN_JS = tuple(int(x) for x in os.environ.get("KERN_DRAINJS", "0,2,5,8").split(","))
STEAL = set(tuple(int(v) for v in x.split("."))
            for x in os.environ.get("KERN_STEAL", "").split(",")
            if x)
EVAC_DEMOTE = int(os.environ.get("KERN_EVDEMOTE", "40"))
SPLIT = set(tuple(int(v) for v in x.split("."))
            for x in os.environ.get("KERN_SPLIT", "").split(",")
            if x)

B, N, D, H, DH, DFF = 4, 2048, 384, 8, 48, 1536
PH = 64            # padded per-head block
DP = H * PH        # 512 padded model dim
ROWS = 1024        # query rows per core
KD8 = D // 128     # 3 in-dim tiles (64 pair-partitions each, DR)
TQ = DP // 128     # 4 head pairs
NJ = N // 128      # 16 key tiles
NI = ROWS // 128   # 8 query i-tiles
NF = DFF // 128    # 12 ffn tiles
KH = DH + 1        # 49 cols per head block incl denominator col
PR = DH // 2       # 24 pair-partitions per head
SCALE = 1.0 / math.sqrt(D)

# Schraudolph fp16 exp: bitcast_f16(int16(s*A16 + B16)) ~= exp(s*SCALE)
A16 = SCALE * 1024.0 / math.log(2.0)
B16 = 15.0 * 1024.0 - 60.0


# DMA segments to compact padded ot_p [512 rows] -> otc [384 rows]:
# (src_tile, src_row, dst_tile, dst_row, nrows)
def _compact_segs():
    segs = []
    for h in range(H):
        s_lo, d, left, off = 64 * (h % 2), DH * h, DH, 0
        while left:
            n = min(left, 128 - ((d + off) % 128))
            segs.append((h // 2, s_lo + off, (d + off) // 128, (d + off) % 128, n))
            off += n
            left -= n
    return segs


CSEGS = _compact_segs()

_CACHE = {}


def _build():
    from contextlib import ExitStack
    import concourse.bass as bass
    import concourse.bacc as bacc
    import concourse.tile as tile
    import concourse.mybir as mybir

    F32 = mybir.dt.float32
    F16 = mybir.dt.float16
    I16 = mybir.dt.int16
    F8 = mybir.dt.float8e4
    AF = mybir.ActivationFunctionType
    ALU = mybir.AluOpType
    DR = mybir.MatmulPerfMode.DoubleRowSwInterleave
    ts = bass.ts

    nc = bacc.Bacc(trn_type="TRN2", target_bir_lowering=False, debug=False)

    def din(name, shape, dt=F16):
        return nc.dram_tensor(name, shape, dt, kind="ExternalInput").ap()

    x8T = din("x8T", [192, 2 * ROWS], F8)
    xT = din("xT", [D, ROWS])
    wqT = din("wqT", [D, DP])
    wq8xT = din("wq8xT", [192, 1536], F8)
    wk8xT = din("wk8xT", [192, 1536], F8)
    y8rT = din("y8rT", [192, 2 * N], F8)
    wv8T = din("wv8T", [192, 2 * D], F8)
    w1T = din("w1T", [D, DFF])
    w2T = din("w2T", [DFF, D])
    idT = din("idT", [128, 128])
    idT32 = din("idT32", [128, 128], F32)
    o = nc.dram_tensor("o", [D, ROWS], F16, kind="ExternalOutput").ap()

    with tile.TileContext(nc) as tc, ExitStack() as ctx:
        sb = ctx.enter_context(tc.tile_pool(name="sb", bufs=1))
        ps = ctx.enter_context(tc.tile_pool(name="ps", bufs=1, space="PSUM"))

        # ---- persistent SBUF tiles ----
        x8 = [sb.tile([64, 2 * ROWS], F8, tag="x8", bufs=3, name=f"x8_{k}")
              for k in range(KD8)]
        # fp16 x / Wq for the qt (residual) projection: the Q residual feeds
        # the output directly (and the FFN input), so fp8 noise there is the
        # single biggest accuracy cost -- keep this one path in fp16
        xt = [sb.tile([128, ROWS], F16, tag="xt", bufs=3, name=f"xt{k}")
              for k in range(KD8)]
        wq = [sb.tile([128, DP], F16, tag="wq", bufs=3, name=f"wq{k}")
              for k in range(KD8)]
        wq8x = [sb.tile([64, 1536], F8, tag="wqx", bufs=3, name=f"wq8x{k}")
                for k in range(KD8)]
        wk8x = [sb.tile([64, 1536], F8, tag="wkx", bufs=3, name=f"wk8x{k}")
                for k in range(KD8)]
        yt8r = [sb.tile([64, 2 * N], F8, tag="ytr", bufs=3, name=f"yt8r_{k}")
                for k in range(KD8)]
        wv8 = [sb.tile([64, 2 * D], F8, tag="wv", bufs=3, name=f"wv8_{k}")
               for k in range(KD8)]
        # 3 heads per group tile at 32-partition boundaries (matmul operand
        # base partition must be 0/32/64): head h -> group h//3, base 32*(h%3)
        GP = [88, 88, 56]
        q8 = [sb.tile([GP[g], 2 * ROWS], F8, tag="q8", bufs=3, name=f"q8_{g}")
              for g in range(3)]
        k8r = [sb.tile([GP[g], 2 * N], F8, tag="k8", bufs=3, name=f"k8r_{g}")
               for g in range(3)]
        qt = [sb.tile([128, ROWS], F16, tag="qt", bufs=4, name=f"qt{t}") for t in range(TQ)]
        vaug = [sb.tile([128, DP], F16, tag="va", bufs=16, name=f"va{j}") for j in range(NJ)]
        ident = sb.tile([128, 128], F16, tag="id", bufs=1, name="ident")
        ident32 = sb.tile([128, 128], F32, tag="id32", bufs=1, name="ident32")
        ot_p = [sb.tile([128, ROWS], F16, tag="otp", bufs=4, name=f"otp{t}") for t in range(TQ)]
        otc = [sb.tile([128, ROWS], F16, tag="otc", bufs=3, name=f"otc{m}") for m in range(KD8)]
        w1 = [sb.tile([128, DFF], F16, tag="w1", bufs=3, name=f"w1_{k}") for k in range(KD8)]
        w2 = [sb.tile([128, D], F16, tag="w2", bufs=12, name=f"w2_{f}") for f in range(NF)]

        # ---- input loads, spread across the SP / gpsimd sequencers (ACT is
        # the evacuation bottleneck, so its queue issues no input DMAs).
        # k8proj(0,*,0) inputs (wk8x + first y8r chunk) land first on SP while
        # q8proj inputs (wq8x + x8) land in parallel on gpsimd. ----
        # critical-path inputs fan out evenly over the 3 DMA queues (SP /
        # gpsimd / ACT, all idle at t=0): everything k8proj(0,*,0) and
        # q8proj(0,*) need lands in ~2.5us
        nc.sync.dma_start(out=wk8x[0][:], in_=wk8xT[ts(0, 64), :])
        nc.gpsimd.dma_start(out=wk8x[1][:], in_=wk8xT[ts(1, 64), :])
        nc.scalar.dma_start(out=wk8x[2][:], in_=wk8xT[ts(2, 64), :])
        nc.sync.dma_start(out=yt8r[0][:, 0:1024], in_=y8rT[ts(0, 64), 0:1024])
        nc.gpsimd.dma_start(out=yt8r[1][:, 0:1024], in_=y8rT[ts(1, 64), 0:1024])
        nc.scalar.dma_start(out=yt8r[2][:, 0:1024], in_=y8rT[ts(2, 64), 0:1024])
        nc.sync.dma_start(out=wq8x[0][:], in_=wq8xT[ts(0, 64), :])
        nc.gpsimd.dma_start(out=wq8x[1][:], in_=wq8xT[ts(1, 64), :])
        nc.scalar.dma_start(out=wq8x[2][:], in_=wq8xT[ts(2, 64), :])
        nc.sync.dma_start(out=x8[0][:], in_=x8T[ts(0, 64), :])
        nc.gpsimd.dma_start(out=x8[1][:], in_=x8T[ts(1, 64), :])
        nc.scalar.dma_start(out=x8[2][:], in_=x8T[ts(2, 64), :])
        for k in range(KD8):
            nc.sync.dma_start(out=yt8r[k][:, 1024:2048], in_=y8rT[ts(k, 64), 1024:2048])
        for k in range(KD8):
            nc.gpsimd.dma_start(out=wv8[k][:], in_=wv8T[ts(k, 64), :])
        nc.sync.dma_start(out=ident[:], in_=idT[:, :])
        nc.sync.dma_start(out=ident32[:], in_=idT32[:, :])
        for k in range(KD8):
            nc.gpsimd.dma_start(out=yt8r[k][:, 2048:4096], in_=y8rT[ts(k, 64), 2048:4096])
        # fp16 qt-projection inputs (needed first at qtproj(0), well after the
        # fp8 warmup projections)
        for k in range(KD8):
            nc.sync.dma_start(out=wq[k][:], in_=wqT[ts(k, 128), :])
            nc.gpsimd.dma_start(out=xt[k][:], in_=xT[ts(k, 128), :])

        def load_ffn_weights():
            for f in range(NF):
                nc.sync.dma_start(out=w2[f][:], in_=w2T[ts(f, 128), :])
            for k in range(KD8):
                nc.sync.dma_start(out=w1[k][:], in_=w1T[ts(k, 128), :])

        # ---- projections ([128,1024] st-tag PSUM tiles) ----
        # Evacuation engine is picked per call to balance ACT vs DVE load in
        # each phase (DVE idles early; ACT is the heavier engine overall).
        from contextlib import contextmanager

        @contextmanager
        def lowprio(off=EVAC_DEMOTE):
            # demote by ~one attention iteration: when an exp tile and a
            # projection evacuation are both ready on an engine, the exp wins
            # (it releases an st-ring slot the whole pipeline waits on); the
            # evacuation fills the next gap instead of blocking
            p0 = tc.cur_priority
            tc.cur_priority = p0 + off
            try:
                yield
            finally:
                tc.cur_priority = p0

        def ev_copy(dve, dst, src):
            with lowprio():
                if dve:
                    nc.vector.tensor_copy(dst, src)
                else:
                    nc.scalar.copy(dst, src)

        def qtproj(t, dve=False, defer=False):
            p = ps.tile([128, 1024], F32, tag="st", bufs=3, name=f"psq{t}")
            for c in range(2):
                for k in range(KD8):
                    nc.tensor.matmul(
                        p[:, ts(c, 512)],
                        wq[k][:, ts(t, 128)], xt[k][:, ts(c, 512)],
                        start=(k == 0), stop=(k == KD8 - 1))
            ev = lambda: ev_copy(dve, qt[t][:], p[:])
            if defer:
                return ev
            ev()

        def q8proj(g, s, dve=True, chunked=False, defer=False):
            p = ps.tile([128, 1024], F32, tag="st", bufs=3, name=f"psq8_{g}_{s}")
            dst = q8[g][:].rearrange("p (n s) -> p s n", s=2)
            for c in range(2):
                for k in range(KD8):
                    nc.tensor.matmul(
                        p[:, ts(c, 512)],
                        wq8x[k][:, 256 * (2 * g + s):256 * (2 * g + s + 1)],
                        x8[k][:, ts(c, 1024)].rearrange("p (n s) -> p s n", s=2),
                        start=(k == 0), stop=(k == KD8 - 1), perf_mode=DR)
                if chunked:
                    # per-chunk evacuation shortens the pipeline-fill critical
                    # path (scores c=0 can start after the first 512 queries)
                    ev_copy(dve, dst[:, s, ts(c, 512)], p[0:GP[g], ts(c, 512)])
            if chunked:
                return
            ev = lambda: ev_copy(dve, dst[:, s, :], p[0:GP[g], :])
            if defer:
                return ev
            ev()

        def k8proj(g, s, half, dve=True, chunked=False, defer=False):
            p = ps.tile([128, 1024], F32, tag="st", bufs=3, name=f"psk8_{g}_{s}_{half}")
            dst = k8r[g][:, ts(half, 2048)].rearrange("p (n s) -> p s n", s=2)
            for c in range(2):
                n = 2 * half + c
                for k in range(KD8):
                    nc.tensor.matmul(
                        p[:, ts(c, 512)],
                        wk8x[k][:, 256 * (2 * g + s):256 * (2 * g + s + 1)],
                        yt8r[k][:, ts(n, 1024)].rearrange("p (n s) -> p s n", s=2),
                        start=(k == 0), stop=(k == KD8 - 1), perf_mode=DR)
                if chunked:
                    ev_copy(dve, dst[:, s, ts(c, 512)], p[0:GP[g], ts(c, 512)])
            if chunked:
                return
            ev = lambda: ev_copy(dve, dst[:, s, :], p[0:GP[g], :])
            if defer:
                return ev
            ev()

        def vproj(j, dve=None):
            p = ps.tile([128, 1024], F32, tag="st", bufs=3, name=f"psv{j}")
            for k in range(KD8):
                nc.tensor.matmul(
                    p[:, 0:D],
                    yt8r[k][:, 256 * j:256 * (j + 1)],
                    wv8[k][:].rearrange("p (n s) -> p s n", s=2),
                    start=(k == 0), stop=(k == KD8 - 1), perf_mode=DR)
            va3 = vaug[j][:].rearrange("p (h e) -> p h e", h=H)
            ps3 = p[:, 0:D].rearrange("p (h e) -> p h e", h=H)
            if dve is None:
                dve = (j < 6) or (j % 2 == 1)
            ev_copy(dve, va3[:, :, 0:DH], ps3[:, :, 0:DH])
            nc.gpsimd.memset(va3[:, :, DH:DH + 1], 1.0)

        # Minimal prologue: only what scores (0, j<4) need. The rest of the
        # projections interleave with the attention loop via SPLIT emission:
        # a projection's matmuls go in one bg slot and its evacuation two
        # slots later, so the evacuation never head-of-line-blocks its engine
        # on pending PE work, and at most ONE parked PSUM tile exists at a
        # time (the st ring needs the other two slots for scores).
        k8proj(0, 0, 0, dve=False, chunked=True)
        k8proj(0, 1, 0, dve=True, chunked=True)
        q8proj(0, 0, dve=False, chunked=True)
        q8proj(0, 1, dve=True, chunked=True)
        vproj(0, dve=False)
        vproj(1, dve=True)

        bg = {}

        def bgadd(slot, fn):
            bg.setdefault(slot, []).append(fn)

        pend_ev = {}

        def mm(slot, key, fn):
            bgadd(slot, lambda: pend_ev.__setitem__(key, fn()))

        def ev(slot, key):
            bgadd(slot, lambda: pend_ev.pop(key)())

        # vprojs: atomic, two slots ahead of first use
        for j in range(2, NJ):
            bgadd((0, j - 2), lambda j=j, d=(j % 2 == 1): vproj(j, dve=d))
        # t=0 needs its second key half by j=8; group-1 k8/q8 by (1,0)
        mm((0, 2), "ka", lambda: k8proj(0, 0, 1, dve=False, defer=True))
        ev((0, 3), "ka")
        mm((0, 4), "kb", lambda: k8proj(0, 1, 1, dve=True, defer=True))
        ev((0, 5), "kb")
        mm((0, 6), "qa", lambda: q8proj(1, 0, dve=False, defer=True))
        ev((0, 7), "qa")
        mm((0, 8), "qb", lambda: q8proj(1, 1, dve=True, defer=True))
        ev((0, 9), "qb")
        mm((0, 10), "kc", lambda: k8proj(1, 0, 0, dve=False, defer=True))
        ev((0, 11), "kc")
        mm((0, 12), "kd", lambda: k8proj(1, 1, 0, dve=True, defer=True))
        ev((0, 13), "kd")
        mm((0, 14), "q0", lambda: qtproj(0, dve=True, defer=True))
        ev((0, 15), "q0")
        mm((1, 1), "ke", lambda: k8proj(1, 0, 1, dve=False, defer=True))
        ev((1, 3), "ke")
        mm((1, 4), "kf", lambda: k8proj(1, 1, 1, dve=True, defer=True))
        ev((1, 6), "kf")
        mm((1, 9), "q1", lambda: qtproj(1, dve=False, defer=True))
        ev((1, 11), "q1")
        mm((1, 12), "qc", lambda: q8proj(2, 0, dve=False, defer=True))
        ev((1, 14), "qc")
        mm((2, 1), "qd", lambda: q8proj(2, 1, dve=True, defer=True))
        ev((2, 3), "qd")
        mm((2, 4), "kg", lambda: k8proj(2, 0, 0, dve=False, defer=True))
        ev((2, 6), "kg")
        mm((2, 7), "kh", lambda: k8proj(2, 1, 0, dve=True, defer=True))
        ev((2, 9), "kh")
        mm((2, 10), "ki", lambda: k8proj(2, 0, 1, dve=False, defer=True))
        ev((2, 12), "ki")
        mm((2, 13), "kj", lambda: k8proj(2, 1, 1, dve=True, defer=True))
        ev((2, 15), "kj")
        mm((3, 0), "q2", lambda: qtproj(2, dve=False, defer=True))
        ev((3, 1), "q2")
        mm((3, 4), "q3", lambda: qtproj(3, dve=True, defer=True))
        ev((3, 6), "q3")
        bgadd((2, 9), load_ffn_weights)

        # ---- attention ----
        # head-B exp tiles stolen by ACT in windows where it has slack
        ACT_STEAL = STEAL
        o_r = {}
        pending = None

        def drain_recips():
            t, accA, accB = pending
            rcs = []
            for a, acc in ((0, accA), (1, accB)):
                # batched reciprocal of the 8 denominator columns (stride 64)
                rc = sb.tile([128, NI], F32, tag="rc", bufs=4, name=f"rc{t}_{a}")
                dens = acc[:].rearrange("p (i e) -> p i e", e=PH)[:, :, DH:DH + 1]
                nc.vector.reciprocal(rc[:], dens)
                rcs.append(rc)
            return rcs

        def drain_norm(rcs, part=None):
            # hybrid norm: head B is ONE stride-0-broadcast
            # scalar_tensor_tensor on DVE (scales all 8 i-blocks at once);
            # head A runs as per-i activation-copies on ACT, which has slack
            # while DVE carries the Schraudolph exps. part=0/1 emits ACT
            # halves separately to spread the burst.
            t, accA, accB = pending
            if part in (None, 0):
                orb = sb.tile([128, 1024], F16, tag="or", bufs=2, name=f"orb{t}")
                o_r[t] = orb
                orb3 = orb[:].rearrange("p (i e) -> p i e", e=128)
                in0 = accB[:].rearrange("p (i e) -> p i e", e=PH)[:, :, 0:KH]
                in1 = rcs[1][:].unsqueeze(2).broadcast_to([128, NI, KH])
                nc.vector.scalar_tensor_tensor(
                    orb3[:, :, PH:PH + KH], in0, 1.0, in1, ALU.mult, ALU.mult)
            orb = o_r[t]
            lo, hi = (0, NI) if part is None else (4 * part, 4 * part + 4)
            for i in range(lo, hi):
                nc.scalar.activation(
                    orb[:, 128 * i:128 * i + KH],
                    accA[:, PH * i:PH * i + KH], AF.Copy,
                    scale=rcs[0][:, i:i + 1])

        def drain_transpose(q):
            t = pending[0]
            # 4 transposes per PSUM tile (one per 2KB bank: no zero-region
            # clobber), then a single fused residual-add evacuates all 4
            tp = ps.tile([128, 1024], F32, tag="st", bufs=3, name=f"tp{t}_{q}")
            for u in range(4):
                tpv = tp[:, 256 * u:256 * u + 64].bitcast(F16)
                i = 4 * q + u
                nc.tensor.transpose(tpv, o_r[t][:, 128 * i:128 * (i + 1)],
                                    ident[:])
            tp4 = tp[:].bitcast(F16).rearrange(
                "p (b r) -> p b r", b=4)[:, :, 0:128]
            nc.vector.scalar_tensor_tensor(
                ot_p[t][:, ts(q, 512)], tp4, 1.0, qt[t][:, ts(q, 512)],
                ALU.mult, ALU.add)

        def drain_csegs(half):
            # column-halved so FFN1's c=0 phase (which reads otc[:, 0:512])
            # unblocks after only the first half of the final drain
            t = pending[0]
            cl, ch = 512 * half, 512 * (half + 1)
            for st_, sr, dt_, dr, nr in CSEGS:
                if st_ == t:
                    nc.gpsimd.dma_start(out=otc[dt_][dr:dr + nr, cl:ch],
                                        in_=ot_p[t][sr:sr + nr, cl:ch])

        def drain_all():
            # final drain: csegs(0) right after transpose(0) -- ot_p cols
            # 0:512 are complete then, which unblocks FFN1's c=0 phase one
            # stt earlier
            drain_norm(drain_recips())
            drain_transpose(0)
            drain_csegs(0)
            drain_transpose(1)
            drain_csegs(1)

        # drain of t's results is spread over t+1's early iterations so the
        # norm/transpose bursts don't stall the exp pipeline
        drain_sched = {}
        for t in range(1, TQ):
            def mk(fn):
                return fn

            # norms at j=0 free the acc ring immediately (P@V of t+1 waits
            # only one iteration); transposes+stt touch only o_rb/ot_p so
            # they can spread wider without acc pressure
            state = {}
            drain_sched[(t, DRAIN_JS[0])] = mk(lambda st=state: (
                st.__setitem__("rcs", drain_recips()),
                drain_norm(st["rcs"], part=0)))
            drain_sched[(t, DRAIN_JS[0] + 1)] = mk(
                lambda st=state: drain_norm(st["rcs"], part=1))
            drain_sched[(t, DRAIN_JS[1])] = mk(lambda: drain_transpose(0))
            drain_sched[(t, DRAIN_JS[2])] = mk(lambda: drain_transpose(1))
            drain_sched[(t, DRAIN_JS[3])] = mk(lambda: (drain_csegs(0),
                                                        drain_csegs(1)))

        for t in range(TQ):
            accA = ps.tile([128, 512], F32, tag="acc", bufs=2, name=f"accA{t}")
            accB = ps.tile([128, 512], F32, tag="acc", bufs=2, name=f"accB{t}")
            for j in range(NJ):
                pe2 = []
                for a in range(2):
                    h = 2 * t + a
                    g, hb = h // 3, 32 * (h % 3)
                    pe = sb.tile([128, 1024], F16, tag="pt", bufs=10,
                                 name=f"pe{t}_{j}_{a}")
                    stx = ps.tile([128, 1024], F32, tag="st", bufs=3,
                                  name=f"st{t}_{j}_{a}")
                    for c in range(2):
                        nc.tensor.matmul(
                            stx[:, ts(c, 512)],
                            k8r[g][hb:hb + PR, 256 * j:256 * (j + 1)],
                            q8[g][hb:hb + PR, ts(c, 1024)]
                                .rearrange("p (n s) -> p s n", s=2),
                            start=True, stop=True, perf_mode=DR)
                    if a == 1 and (t, j) in SPLIT:
                        # split the head-B exp across both engines: DVE (the
                        # steady-state pacer) gets a half-iteration of relief
                        # without an ACT double-burst
                        nc.scalar.activation(pe[:, 0:512], stx[:, 0:512],
                                             AF.Exp, scale=SCALE)
                        nc.vector.tensor_scalar(
                            pe[:, 512:1024].bitcast(I16), stx[:, 512:1024],
                            A16, B16, ALU.mult, ALU.add)
                    elif a == 1 and (t, j) not in ACT_STEAL:
                        nc.vector.tensor_scalar(
                            pe[:].bitcast(I16), stx[:], A16, B16,
                            ALU.mult, ALU.add)
                    else:
                        nc.scalar.activation(pe[:], stx[:], AF.Exp, scale=SCALE)
                    pe2.append(pe)
                for fn in bg.get((t, j), ()):
                    fn()
                if pending is not None and (t, j) in drain_sched:
                    drain_sched[(t, j)]()
                for a, acc in ((0, accA), (1, accB)):
                    for i in range(NI):
                        nc.tensor.matmul(
                            acc[:, PH * i:PH * i + KH],
                            pe2[a][:, ts(i, 128)],
                            vaug[j][:, PH * (2 * t + a):PH * (2 * t + a) + KH],
                            start=(j == 0 and i == 0),
                            stop=(j == NJ - 1),
                            skip_group_check=True)
            pending = (t, accA, accB)
        drain_all()

        # ---- FFN (feature-major, compact): otc -> gelu(W1@otc) -> W2@hid + otc
        # (fp16 hidden/w2: an fp8-DR FFN2 was tried and costs ~2.3% output
        # error -- hidden*w2 quantization noise does not average out relative
        # to the FFN output, unlike all the attention-path fp8.)
        for c in range(2):
            # FFN2 accumulators m=0,1 are fed as soon as each gelu tile lands;
            # m=2 accumulates after the g-loop from retained hf tiles, so only
            # its 12 matmuls + add trail the final gelu.
            po = [ps.tile([128, 512], F32, tag="acc", bufs=2, name=f"po{c}_{m}")
                  for m in range(2)]
            hf = []
            for g in range(NF // 2):
                sg = ps.tile([128, 1024], F32, tag="st", bufs=3, name=f"sg{c}_{g}")
                for fi in range(2):
                    for k in range(KD8):
                        nc.tensor.matmul(
                            sg[:, ts(fi, 512)],
                            w1[k][:, ts(g * 2 + fi, 128)], otc[k][:, ts(c, 512)],
                            start=(k == 0), stop=(k == KD8 - 1))
                h = sb.tile([128, 1024], F16, tag="hid", bufs=10, name=f"hf{c}_{g}")
                nc.scalar.activation(h[:], sg[:], AF.Gelu)
                hf.append(h)
                for m in range(2):
                    for fi in range(2):
                        nc.tensor.matmul(
                            po[m][:], w2[g * 2 + fi][:, ts(m, 128)],
                            h[:, ts(fi, 512)],
                            start=(g == 0 and fi == 0),
                            stop=(g == NF // 2 - 1 and fi == 1))
            po2 = ps.tile([128, 1024], F32, tag="st", bufs=3, name=f"po2_{c}")
            for g in range(NF // 2):
                for fi in range(2):
                    nc.tensor.matmul(
                        po2[:, 0:512], w2[g * 2 + fi][:, 256:384],
                        hf[g][:, ts(fi, 512)],
                        start=(g == 0 and fi == 0),
                        stop=(g == NF // 2 - 1 and fi == 1))
            for m in range(2):
                osb = sb.tile([128, 512], F16, tag="osb", bufs=6, name=f"osb{c}_{m}")
                nc.vector.tensor_add(osb[:], po[m][:], otc[m][:, ts(c, 512)])
                nc.sync.dma_start(out=o[ts(m, 128), ts(c, 512)], in_=osb[:])
            # last chain's add + store in halves so the first half's DMA
            # overlaps the second half's add (shorter kernel tail)
            for hh in range(2):
                osb = sb.tile([128, 256], F16, tag="osb2", bufs=4,
                              name=f"osb2_{c}_{hh}")
                nc.vector.tensor_add(
                    osb[:], po2[:, 256 * hh:256 * hh + 256],
                    otc[2][:, 512 * c + 256 * hh:512 * c + 256 * hh + 256])
                nc.sync.dma_start(
                    out=o[256:384, 512 * c + 256 * hh:512 * c + 256 * hh + 256],
                    in_=osb[:])

    nc.compile()
    return nc


def _interleave_rows(w):
    # [R, X] -> [R/2, 2X]: out[64k+p, 2n+s] = w[128k+2p+s, n]
    x = w.reshape(-1, 64, 2, w.shape[1])        # [k, p, s, n]
    x = np.transpose(x, (0, 1, 3, 2))           # [k, p, n, s]
    return np.ascontiguousarray(x.reshape(w.shape[0] // 2, -1))


def _interleave_rows_rev(w):
    # as _interleave_rows, but columns reversed within each 128-col block
    # (DoubleRowSwInterleave weight layout)
    wr = np.ascontiguousarray(
        w.reshape(w.shape[0], -1, 128)[:, :, ::-1]).reshape(w.shape[0], -1)
    return _interleave_rows(wr)


def _pad_rows(w):  # [384, X] -> [512, X]; head h dims at rows 64h..64h+47
    out = np.zeros((DP,) + w.shape[1:], dtype=w.dtype)
    out.reshape(H, PH, -1)[:, 0:DH] = w.reshape(H, DH, -1)
    return out


def _parity_weights(Wm):
    # Wm [384 out-feats, 384 in] -> [384 in, 768] with (g, s) blocks of 128
    # cols: col 128*(2g+s) + 32*hh + p = Wm row (head 3g+hh, dim 2p+s), p<24.
    WT = Wm.T  # [in, feat]
    blocks = []
    for g in range(3):
        for s in range(2):
            Wg = np.zeros((D, 128), np.float32)
            for hh in range(3):
                h = 3 * g + hh
                if h >= H:
                    continue
                for p in range(PR):
                    Wg[:, 32 * hh + p] = WT[:, h * DH + 2 * p + s]
            blocks.append(Wg)
    return np.concatenate(blocks, axis=1)


def _prep_weights(Wq, Wk, Wv, W1, W2):
    import ml_dtypes
    f8 = ml_dtypes.float8_e4m3fn
    f16 = np.float16
    wqT = np.ascontiguousarray(_pad_rows(Wq).T).astype(f16)   # [384, 512]
    wq8xT = _interleave_rows_rev(_parity_weights(Wq)).astype(f8)  # [192, 1536]
    wk8xT = _interleave_rows_rev(_parity_weights(Wk)).astype(f8)  # [192, 1536]
    wv8T = _interleave_rows(np.ascontiguousarray(Wv.T)).astype(f8)
    w1T = np.ascontiguousarray(W1.T).astype(f16)              # [384, 1536]
    w2T = np.ascontiguousarray(W2.T).astype(f16)              # [1536, 384]
    return wqT, wq8xT, wk8xT, wv8T, w1T, w2T


def _run(in_maps, trace=False):
    from concourse.bass_utils import run_bass_kernel_spmd

    if "nc" not in _CACHE:
        _CACHE["nc"] = _build()
    try:
        return run_bass_kernel_spmd(_CACHE["nc"], in_maps, list(range(8)), trace=trace)
    except Exception:
        # one retry: absorbs transient device wedges (NRT_EXEC_UNIT_* from a
        # previous interrupted run on the shared tunneled devices). Once PJRT
        # marks a device unrecoverable the client is poisoned, so drop the
        # cached backends to force a fresh client before retrying.
        import time as _time
        last = None
        for delay in (10.0, 30.0):
            try:
                import jax
                import jax._src.xla_bridge as _xb
                jax.clear_caches()
                with _xb._backend_lock:
                    _xb._backends.clear()
                    _xb._backend_errors.clear()
            except Exception:
                pass
            _time.sleep(delay)
            try:
                return run_bass_kernel_spmd(_CACHE["nc"], in_maps,
                                            list(range(8)), trace=trace)
            except Exception as e:  # noqa
                last = e
        raise last


def _make_in_maps(x, y, Wq, Wk, Wv, W1, W2):
    import ml_dtypes
    f8d = ml_dtypes.float8_e4m3fn
    x = np.asarray(x, dtype=np.float32)
    y = np.asarray(y, dtype=np.float32)
    wqT, wq8xT, wk8xT, wv8T, w1T, w2T = _prep_weights(
        np.asarray(Wq, np.float32), np.asarray(Wk, np.float32),
        np.asarray(Wv, np.float32), np.asarray(W1, np.float32),
        np.asarray(W2, np.float32))
    ident = np.eye(128, dtype=np.float16)
    in_maps = []
    for c in range(8):
        b, half = c // 2, c % 2
        xs = x[b, half * ROWS:(half + 1) * ROWS]  # [1024, 384]
        xsT = np.ascontiguousarray(xs.T)
        yTc = np.ascontiguousarray(y[b].T)
        m = {
            "x8T": _interleave_rows(xsT.astype(f8d)),
            "xT": xsT.astype(np.float16),
            "y8rT": _interleave_rows_rev(yTc.astype(f8d)),
            "wqT": wqT, "wq8xT": wq8xT, "wk8xT": wk8xT, "wv8T": wv8T,
            "w1T": w1T, "w2T": w2T, "idT": ident,
            "idT32": ident.astype(np.float32),
        }
        in_maps.append(m)
    return in_maps


def _unshard(results):
    out = np.empty((B, N, D), np.float32)
    for c in range(8):
        oc = np.asarray(results[c]["o"], np.float32)  # [384, 1024] fp16 on dev
        out[c // 2, (c % 2) * ROWS:(c % 2 + 1) * ROWS, :] = oc.T
    return out


def kernel(x, y, Wq, Wk, Wv, W1, W2):
    res = _run(_make_in_maps(x, y, Wq, Wk, Wv, W1, W2))
    return _unshard(res.results)


def profile(x, y, Wq, Wk, Wv, W1, W2):
    """Run with NTFF tracing; returns exec_time_ns (or None)."""
    import concourse.bass_utils as bu
    orig = bu.upload_artifacts
    bu.upload_artifacts = lambda tmpdir: f"file://{tmpdir}"
    try:
        res = _run(_make_in_maps(x, y, Wq, Wk, Wv, W1, W2), trace=True)
    finally:
        bu.upload_artifacts = orig
    return res.exec_time_ns
